# revision 13
# baseline (speedup 1.0000x reference)
"""Trainium2 Bass kernel for nn_CrossAttention (B=8, C=256, W=H=64).

Sharding: data-parallel over batch across the 8 NeuronCores; the small 1x1
conv weights and gamma are replicated.

Per-core computation (one batch, xf = x reshaped [C, N], N = W*H = 4096):
    f   = Wf @ xf            [CQ, N]   (bf16 matmul, f32 psum)
    g   = Wg @ yf            [CQ, N]
    hhT = (Wh @ xf + bh).T   [N, C]    (stored transposed, + ones column)
    LT[j, i]  = sum_d g[d, j] f[d, i]          (transposed logits)
    E = exp(LT)  (no max subtraction: logits are O(60) max, exp fits f32/bf16)
    UT[i, c] = sum_j E[j, i] * hhT[j, c]   -> col C holds D[i] = sum_j E[j, i]
    satT[i, c] = UT[i, c] * gamma / D[i]
    out[c, i] = satT.T + x[c, i]

v2 scheduling notes:
  - logit PSUM is double buffered as [128, 2, IC] pair tiles (2 banks x 2
    bufs) and exp runs as one ACT instruction per pair, so the activation
    engine stays ahead of the PE instead of gating it.
  - logits for group jg+1 are emitted before the UT matmuls of group jg
    (software pipelining), including across i-chunk boundaries.
  - bh is added on the vector engine from a broadcast copy, not via K=1
    PE matmuls.
  - inputs are loaded in 512-column chunks with the f/g projections
    following each chunk, so the PE starts as soon as the first chunk lands.
"""

import numpy as np

import concourse.bass as bass
import concourse.mybir as mybir
import concourse.tile as tile
from concourse import bacc
from concourse.bass import ds, ts
from concourse.bass_utils import run_bass_kernel_spmd
from concourse.masks import make_identity

FP32 = mybir.dt.float32
BF16 = mybir.dt.bfloat16
AF = mybir.ActivationFunctionType
ALU = mybir.AluOpType

C = 256
CQ = 32
N_CORES = 8


def build_nc(n=4096):
    """Build the single-core Bass program (SPMD across cores via inputs)."""
    assert n % 128 == 0
    NB = n // 128            # number of 128-row j blocks
    IC = 512                 # i-chunk size for the main loop
    NIC = n // IC            # number of i chunks
    NQ = IC // 128           # 128-row i tiles per chunk
    JQ = 4                   # j blocks per PE row-tiling group
    NJG = NB // JQ           # j groups per i chunk
    DCH = 2048               # input dma chunk cols (8KB rows: full DMA rate)
    NDCH = n // DCH
    FCH = 512                # f/g projection sub-chunk cols

    nc = bacc.Bacc("TRN2", target_bir_lowering=False, debug=False)

    x_d = nc.dram_tensor("x", [C, n], FP32, kind="ExternalInput").ap()
    y_d = nc.dram_tensor("y", [C, n], FP32, kind="ExternalInput").ap()
    wfT_d = nc.dram_tensor("wfT", [C, CQ], FP32, kind="ExternalInput").ap()
    wgT_d = nc.dram_tensor("wgT", [C, CQ], FP32, kind="ExternalInput").ap()
    whT_d = nc.dram_tensor("whT", [C, C], FP32, kind="ExternalInput").ap()
    bf_d = nc.dram_tensor("bf", [CQ, 1], FP32, kind="ExternalInput").ap()
    bg_d = nc.dram_tensor("bg", [CQ, 1], FP32, kind="ExternalInput").ap()
    bh_d = nc.dram_tensor("bh", [1, C], FP32, kind="ExternalInput").ap()
    gamma_d = nc.dram_tensor("gamma", [1, 1], FP32, kind="ExternalInput").ap()
    out_d = nc.dram_tensor("out", [C, n], FP32, kind="ExternalOutput").ap()

    with tile.TileContext(nc) as tc:
        with tc.tile_pool(name="persist", bufs=1) as persist, \
             tc.tile_pool(name="consts", bufs=1) as consts:
            # ---- persistent SBUF tensors -------------------------------
            x_sb = persist.tile([128, 2, n], FP32, tag="x_sb")
            y_sb = persist.tile([128, 2, n], FP32, tag="y_sb")
            xb = persist.tile([128, 2, n], BF16, tag="xb")
            yb = persist.tile([128, 2, n], BF16, tag="yb")
            f_rep = persist.tile([128, n], BF16, tag="f_rep")   # 4 replicated bands
            g_rep = persist.tile([128, n], BF16, tag="g_rep")
            hhT = persist.tile([128, NB, C + 1], BF16, tag="hhT")

            wfT_f = consts.tile([128, 2, CQ], FP32, tag="wfT_f")
            wgT_f = consts.tile([128, 2, CQ], FP32, tag="wgT_f")
            whT_f = consts.tile([128, 2, C], FP32, tag="whT_f")
            wfT_b = consts.tile([128, 2, CQ], BF16, tag="wfT_b")
            wgT_b = consts.tile([128, 2, CQ], BF16, tag="wgT_b")
            whT_b = consts.tile([128, 2, C], BF16, tag="whT_b")
            bf_sb = consts.tile([CQ, 1], FP32, tag="bf_sb")
            bg_sb = consts.tile([CQ, 1], FP32, tag="bg_sb")
            bh_bc = consts.tile([128, C], FP32, tag="bh_bc")
            gamma_sb = consts.tile([128, 1], FP32, tag="gamma_sb")
            ident = consts.tile([128, 128], BF16, tag="ident")
            warm = consts.tile([CQ, 1], FP32, tag="warm")

            # ---- weight DMAs + casts (small, first) --------------------
            for cb in range(2):
                nc.sync.dma_start(out=wfT_f[:, cb, :], in_=wfT_d[cb * 128:(cb + 1) * 128, :])
                nc.sync.dma_start(out=wgT_f[:, cb, :], in_=wgT_d[cb * 128:(cb + 1) * 128, :])
                nc.sync.dma_start(out=whT_f[:, cb, :], in_=whT_d[cb * 128:(cb + 1) * 128, :])
            nc.sync.dma_start(out=bf_sb, in_=bf_d[:, :])
            nc.sync.dma_start(out=bg_sb, in_=bg_d[:, :])
            nc.sync.dma_start(out=bh_bc, in_=bh_d[:, :].to_broadcast([128, C]))
            nc.sync.dma_start(out=gamma_sb, in_=gamma_d[:, :].to_broadcast([128, 1]))
            for cb in range(2):
                nc.vector.tensor_copy(wfT_b[:, cb, :], wfT_f[:, cb, :])
                nc.vector.tensor_copy(wgT_b[:, cb, :], wgT_f[:, cb, :])
                nc.vector.tensor_copy(whT_b[:, cb, :], whT_f[:, cb, :])
            make_identity(nc, ident)
            # force the ACT exp table load off the critical path
            nc.scalar.activation(warm, bf_sb, AF.Exp)

            # one PSUM pool pair for the whole kernel (no pool-transition
            # sync between the projection phase and the attention loop):
            # lt 2 bufs x 2 banks + ut 4 bufs x 1 bank = 8 banks.
            with tc.tile_pool(name="lt", bufs=2, space="PSUM") as ltp, \
                 tc.tile_pool(name="ut", bufs=4, space="PSUM") as utp, \
                 tc.tile_pool(name="ex", bufs=4) as exp_pool, \
                 tc.tile_pool(name="tail", bufs=8) as tailp, \
                 tc.tile_pool(name="stage", bufs=4) as stagep:

                # ---- input DMA issue (no compute deps: the sync engine
                # must not stall mid-stream or later chunks are delayed) --
                for ch in range(NDCH):
                    for cb in range(2):
                        nc.sync.dma_start(out=x_sb[:, cb, ts(ch, DCH)],
                                          in_=x_d[cb * 128:(cb + 1) * 128, ts(ch, DCH)])
                        nc.sync.dma_start(out=y_sb[:, cb, ts(ch, DCH)],
                                          in_=y_d[cb * 128:(cb + 1) * 128, ts(ch, DCH)])
                # ---- cast + f/g/hh as data arrives ---------------------
                for ch in range(NDCH):
                    for cb in range(2):
                        nc.vector.tensor_copy(xb[:, cb, ts(ch, DCH)], x_sb[:, cb, ts(ch, DCH)])
                        nc.vector.tensor_copy(yb[:, cb, ts(ch, DCH)], y_sb[:, cb, ts(ch, DCH)])
                    for sc in range(DCH // FCH):
                        fc = ch * (DCH // FCH) + sc
                        pf = ltp.tile([CQ, FCH], FP32, tag="lt", name="pf")
                        for cb in range(2):
                            nc.tensor.matmul(pf, lhsT=wfT_b[:, cb, :],
                                             rhs=xb[:, cb, ts(fc, FCH)],
                                             start=(cb == 0), stop=(cb == 1))
                        nc.vector.tensor_scalar_add(f_rep[0:CQ, ts(fc, FCH)], pf, bf_sb)
                        pg = ltp.tile([CQ, FCH], FP32, tag="lt", name="pg")
                        for cb in range(2):
                            nc.tensor.matmul(pg, lhsT=wgT_b[:, cb, :],
                                             rhs=yb[:, cb, ts(fc, FCH)],
                                             start=(cb == 0), stop=(cb == 1))
                        nc.vector.tensor_scalar_add(g_rep[0:CQ, ts(fc, FCH)], pg, bg_sb)
                    # hhT for the j blocks covered by this chunk
                    for jb in range(ch * (DCH // 128), (ch + 1) * (DCH // 128)):
                        ph = utp.tile([128, C], FP32, tag="ut", name="ph")
                        nc.tensor.matmul(ph, lhsT=xb[:, 0, ts(jb, 128)], rhs=whT_b[:, 0, :],
                                         start=True, stop=False)
                        nc.tensor.matmul(ph, lhsT=xb[:, 1, ts(jb, 128)], rhs=whT_b[:, 1, :],
                                         start=False, stop=True)
                        nc.vector.tensor_add(hhT[:, jb, 0:C], ph, bh_bc)
                        nc.vector.memset(hhT[:, jb, C:C + 1], 1.0)
                # replicate the f/g bands in two halves (issued after all
                # input DMAs so the sync engine never stalls input issue)
                for h0 in (0, n // 2):
                    for r in range(1, 4):
                        nc.sync.dma_start(out=f_rep[32 * r:32 * (r + 1), ds(h0, n // 2)],
                                          in_=f_rep[0:32, ds(h0, n // 2)])
                        nc.sync.dma_start(out=g_rep[32 * r:32 * (r + 1), ds(h0, n // 2)],
                                          in_=g_rep[0:32, ds(h0, n // 2)])

                # ---- main attention loop -------------------------------

                def emit_pair(ic, jg, pr):
                    """2 row-packed logit matmuls into one [128, 2, IC]
                    psum pair tile + the exp over the pair; returns the bf16
                    exp tile covering jj = 2*pr, 2*pr+1."""
                    lt = ltp.tile([128, 2, IC], FP32, tag="lt",
                                  name=f"lt{ic}_{jg}_{pr}")
                    for h in range(2):
                        jj = pr * 2 + h
                        j = jg * JQ + jj
                        nc.tensor.matmul(
                            lt[:, h, :],
                            lhsT=g_rep[32 * jj:32 * (jj + 1), ts(j, 128)],
                            rhs=f_rep[32 * jj:32 * (jj + 1), ds(ic * IC, IC)],
                            start=True, stop=True,
                            tile_position=(32 * jj, 0))
                    ex = exp_pool.tile([128, 2, IC], BF16, tag="ex",
                                       name=f"ex{ic}_{jg}_{pr}")
                    nc.scalar.activation(ex, lt, AF.Exp)
                    return ex

                def emit_ut(uts, exs, ic, jg, qs):
                    for q in qs:
                        for jj in range(JQ):
                            j = jg * JQ + jj
                            nc.tensor.matmul(
                                uts[q][:, 0:C + 1],
                                lhsT=exs[jj // 2][:, jj % 2, ds(q * 128, 128)],
                                rhs=hhT[:, j, :],
                                start=(jg == 0 and jj == 0),
                                stop=(jg == NJG - 1 and jj == JQ - 1),
                                skip_group_check=True)

                pending = {}
                for ic in range(NIC):
                    uts = [utp.tile([128, 512], FP32, tag="ut", name=f"ut{q}")
                           for q in range(NQ)]
                    if ic == 0:
                        pending[(0, 0)] = [emit_pair(0, 0, 0), emit_pair(0, 0, 1)]
                    for jg in range(NJG):
                        nxt = (ic, jg + 1) if jg + 1 < NJG else \
                              ((ic + 1, 0) if ic + 1 < NIC else None)
                        # pair0 of the next group ahead of this group's UT;
                        # pair1 in the middle, so its exp lands just before
                        # the next group's UT needs it (keeps ACT off the
                        # critical chain)
                        if nxt is not None:
                            pending[nxt] = [emit_pair(nxt[0], nxt[1], 0)]
                        exs = pending.pop((ic, jg))
                        emit_ut(uts, exs, ic, jg, (0, 1))
                        if nxt is not None:
                            pending[nxt].append(emit_pair(nxt[0], nxt[1], 1))
                        emit_ut(uts, exs, ic, jg, (2, 3))
                    # tail: per q, normalize (freeing that ut psum bank),
                    # then emit the previous q's transpose/residual/store so
                    # the "ut" slot WAR chain resolves early and the PE is
                    # never starved across the i-chunk boundary.
                    satTs = []

                    def emit_xpose(qq):
                        i0 = ic * IC + qq * 128
                        stage = stagep.tile([128, 2, 128], FP32, tag="stage",
                                            name="stage")
                        tp = utp.tile([128, 128], BF16, tag="ut", name="tp")
                        for cb in range(2):
                            nc.tensor.transpose(tp, satTs[qq][:, ds(cb * 128, 128)],
                                                ident)
                            nc.vector.tensor_add(stage[:, cb, :], tp,
                                                 x_sb[:, cb, ds(i0, 128)])
                            nc.sync.dma_start(
                                out=out_d[cb * 128:(cb + 1) * 128, ds(i0, 128)],
                                in_=stage[:, cb, :])

                    for q in range(NQ):
                        rd = tailp.tile([128, 1], FP32, tag="rd")
                        nc.vector.reciprocal(rd, uts[q][:, C:C + 1])
                        satT = tailp.tile([128, C], BF16, tag="satT")
                        nc.vector.tensor_scalar(satT, uts[q][:, 0:C], rd, gamma_sb,
                                                op0=ALU.mult, op1=ALU.mult)
                        satTs.append(satT)
                        if q >= 1:
                            emit_xpose(q - 1)
                    emit_xpose(NQ - 1)

    nc.compile()
    return nc


_NC_CACHE = {}


def _get_nc(n=4096):
    if n not in _NC_CACHE:
        _NC_CACHE[n] = build_nc(n)
    return _NC_CACHE[n]


def make_in_maps(x, y, Wf, bf, Wg, bg, Wh, bh, gamma):
    x = np.asarray(x, dtype=np.float32)
    y = np.asarray(y, dtype=np.float32)
    B, C_, W_, H_ = x.shape
    n = W_ * H_
    wfT = np.ascontiguousarray(np.asarray(Wf, np.float32).T)
    wgT = np.ascontiguousarray(np.asarray(Wg, np.float32).T)
    whT = np.ascontiguousarray(np.asarray(Wh, np.float32).T)
    bf_ = np.asarray(bf, np.float32).reshape(CQ, 1)
    bg_ = np.asarray(bg, np.float32).reshape(CQ, 1)
    bh_ = np.asarray(bh, np.float32).reshape(1, C_)
    gm_ = np.asarray(gamma, np.float32).reshape(1, 1)
    in_maps = []
    for b in range(B):
        in_maps.append({
            "x": np.ascontiguousarray(x[b].reshape(C_, n)),
            "y": np.ascontiguousarray(y[b].reshape(C_, n)),
            "wfT": wfT, "wgT": wgT, "whT": whT,
            "bf": bf_, "bg": bg_, "bh": bh_, "gamma": gm_,
        })
    return in_maps, (B, C_, W_, H_)


def run_spmd(inputs: dict, trace: bool = False):
    """Run the SPMD kernel; returns (out [B,C,W,H], BassKernelResults)."""
    in_maps, (B, C_, W_, H_) = make_in_maps(**inputs)
    nc = _get_nc(W_ * H_)
    res = run_bass_kernel_spmd(nc, in_maps, core_ids=list(range(B)), trace=trace)
    out = np.stack([res.results[b]["out"].reshape(C_, W_, H_) for b in range(B)])
    return np.ascontiguousarray(out, dtype=np.float32), res


def kernel(x, y, Wf, bf, Wg, bg, Wh, bh, gamma):
    out, _ = run_spmd(dict(x=x, y=y, Wf=Wf, bf=bf, Wg=Wg, bg=bg,
                           Wh=Wh, bh=bh, gamma=gamma))
    return out


# revision 16
# speedup vs baseline: 1.0639x; 1.0639x over previous
"""Trainium2 Bass kernel for nn_CrossAttention (B=8, C=256, W=H=64).

Sharding: data-parallel over batch across the 8 NeuronCores; the small 1x1
conv weights and gamma are replicated.

Per-core computation (one batch, xf = x reshaped [C, N], N = W*H = 4096):
    f   = Wf @ xf            [CQ, N]   (bf16 matmul, f32 psum)
    g   = Wg @ yf            [CQ, N]
    hhT = (Wh @ xf + bh).T   [N, C]    (stored transposed, + ones column)
    LT[j, i]  = sum_d g[d, j] f[d, i]          (transposed logits)
    E = exp(LT)  (no max subtraction: logits are O(60) max, exp fits f32/bf16)
    UT[i, c] = sum_j E[j, i] * hhT[j, c]   -> col C holds D[i] = sum_j E[j, i]
    satT[i, c] = UT[i, c] * gamma / D[i]
    out[c, i] = satT.T + x[c, i]

v2 scheduling notes:
  - logit PSUM is double buffered as [128, 2, IC] pair tiles (2 banks x 2
    bufs) and exp runs as one ACT instruction per pair, so the activation
    engine stays ahead of the PE instead of gating it.
  - logits for group jg+1 are emitted before the UT matmuls of group jg
    (software pipelining), including across i-chunk boundaries.
  - bh is added on the vector engine from a broadcast copy, not via K=1
    PE matmuls.
  - inputs are loaded in 512-column chunks with the f/g projections
    following each chunk, so the PE starts as soon as the first chunk lands.
"""

import numpy as np

import concourse.bass as bass
import concourse.mybir as mybir
import concourse.tile as tile
from concourse import bacc
from concourse.bass import ds, ts
from concourse.bass_utils import run_bass_kernel_spmd
from concourse.masks import make_identity

FP32 = mybir.dt.float32
BF16 = mybir.dt.bfloat16
AF = mybir.ActivationFunctionType
ALU = mybir.AluOpType

C = 256
CQ = 32
N_CORES = 8


def build_nc(n=4096):
    """Build the single-core Bass program (SPMD across cores via inputs)."""
    assert n % 128 == 0
    NB = n // 128            # number of 128-row j blocks
    IC = 512                 # i-chunk size for the main loop
    NIC = n // IC            # number of i chunks
    NQ = IC // 128           # 128-row i tiles per chunk
    JQ = 4                   # j blocks per PE row-tiling group
    NJG = NB // JQ           # j groups per i chunk
    DCH = 2048               # input dma chunk cols (8KB rows: full DMA rate)
    NDCH = n // DCH
    FCH = 512                # f/g projection sub-chunk cols

    nc = bacc.Bacc("TRN2", target_bir_lowering=False, debug=False)

    x_d = nc.dram_tensor("x", [C, n], FP32, kind="ExternalInput").ap()
    y_d = nc.dram_tensor("y", [C, n], FP32, kind="ExternalInput").ap()
    wfT_d = nc.dram_tensor("wfT", [C, CQ], FP32, kind="ExternalInput").ap()
    wgT_d = nc.dram_tensor("wgT", [C, CQ], FP32, kind="ExternalInput").ap()
    whT_d = nc.dram_tensor("whT", [C, C], FP32, kind="ExternalInput").ap()
    bf_d = nc.dram_tensor("bf", [CQ, 1], FP32, kind="ExternalInput").ap()
    bg_d = nc.dram_tensor("bg", [CQ, 1], FP32, kind="ExternalInput").ap()
    bh_d = nc.dram_tensor("bh", [1, C], FP32, kind="ExternalInput").ap()
    gamma_d = nc.dram_tensor("gamma", [1, 1], FP32, kind="ExternalInput").ap()
    out_d = nc.dram_tensor("out", [C, n], FP32, kind="ExternalOutput").ap()

    with tile.TileContext(nc) as tc:
        with tc.tile_pool(name="persist", bufs=1) as persist, \
             tc.tile_pool(name="consts", bufs=1) as consts:
            # ---- persistent SBUF tensors -------------------------------
            x_sb = persist.tile([128, 2, n], FP32, tag="x_sb")
            y_sb = persist.tile([128, 2, n], FP32, tag="y_sb")
            xb = persist.tile([128, 2, n], BF16, tag="xb")
            yb = persist.tile([128, 2, n], BF16, tag="yb")
            f_rep = persist.tile([128, n], BF16, tag="f_rep")   # 4 replicated bands
            g_rep = persist.tile([128, n], BF16, tag="g_rep")
            hhT = persist.tile([128, NB, C + 1], BF16, tag="hhT")

            wfT_f = consts.tile([128, 2, CQ], FP32, tag="wfT_f")
            wgT_f = consts.tile([128, 2, CQ], FP32, tag="wgT_f")
            whT_f = consts.tile([128, 2, C], FP32, tag="whT_f")
            wfT_b = consts.tile([128, 2, CQ], BF16, tag="wfT_b")
            wgT_b = consts.tile([128, 2, CQ], BF16, tag="wgT_b")
            whT_b = consts.tile([128, 2, C], BF16, tag="whT_b")
            bf_sb = consts.tile([CQ, 1], FP32, tag="bf_sb")
            bg_sb = consts.tile([CQ, 1], FP32, tag="bg_sb")
            bh_bc = consts.tile([128, C], FP32, tag="bh_bc")
            gamma_sb = consts.tile([128, 1], FP32, tag="gamma_sb")
            ident = consts.tile([128, 128], BF16, tag="ident")
            warm = consts.tile([CQ, 1], FP32, tag="warm")

            # ---- weight DMAs + casts (small, first) --------------------
            for cb in range(2):
                nc.sync.dma_start(out=wfT_f[:, cb, :], in_=wfT_d[cb * 128:(cb + 1) * 128, :])
                nc.sync.dma_start(out=wgT_f[:, cb, :], in_=wgT_d[cb * 128:(cb + 1) * 128, :])
                nc.sync.dma_start(out=whT_f[:, cb, :], in_=whT_d[cb * 128:(cb + 1) * 128, :])
            nc.sync.dma_start(out=bf_sb, in_=bf_d[:, :])
            nc.sync.dma_start(out=bg_sb, in_=bg_d[:, :])
            nc.sync.dma_start(out=bh_bc, in_=bh_d[:, :].to_broadcast([128, C]))
            nc.sync.dma_start(out=gamma_sb, in_=gamma_d[:, :].to_broadcast([128, 1]))
            for cb in range(2):
                nc.vector.tensor_copy(wfT_b[:, cb, :], wfT_f[:, cb, :])
                nc.vector.tensor_copy(wgT_b[:, cb, :], wgT_f[:, cb, :])
                nc.vector.tensor_copy(whT_b[:, cb, :], whT_f[:, cb, :])
            make_identity(nc, ident)
            # force the ACT exp table load off the critical path
            nc.scalar.activation(warm, bf_sb, AF.Exp)

            # one PSUM pool pair for the whole kernel (no pool-transition
            # sync between the projection phase and the attention loop):
            # lt 2 bufs x 2 banks + ut 4 bufs x 1 bank = 8 banks.
            with tc.tile_pool(name="lt", bufs=2, space="PSUM") as ltp, \
                 tc.tile_pool(name="ut", bufs=4, space="PSUM") as utp, \
                 tc.tile_pool(name="ex", bufs=8) as exp_pool, \
                 tc.tile_pool(name="tail", bufs=8) as tailp, \
                 tc.tile_pool(name="stage", bufs=4) as stagep:

                # ---- input DMA issue (no compute deps: the sync engine
                # must not stall mid-stream or later chunks are delayed) --
                for ch in range(NDCH):
                    for cb in range(2):
                        nc.sync.dma_start(out=x_sb[:, cb, ts(ch, DCH)],
                                          in_=x_d[cb * 128:(cb + 1) * 128, ts(ch, DCH)])
                        nc.sync.dma_start(out=y_sb[:, cb, ts(ch, DCH)],
                                          in_=y_d[cb * 128:(cb + 1) * 128, ts(ch, DCH)])
                # ---- cast + f/g/hh as data arrives ---------------------
                for ch in range(NDCH):
                    for cb in range(2):
                        nc.vector.tensor_copy(xb[:, cb, ts(ch, DCH)], x_sb[:, cb, ts(ch, DCH)])
                        nc.vector.tensor_copy(yb[:, cb, ts(ch, DCH)], y_sb[:, cb, ts(ch, DCH)])
                    for sc in range(DCH // FCH):
                        fc = ch * (DCH // FCH) + sc
                        pf = ltp.tile([CQ, FCH], FP32, tag="lt", name="pf")
                        for cb in range(2):
                            nc.tensor.matmul(pf, lhsT=wfT_b[:, cb, :],
                                             rhs=xb[:, cb, ts(fc, FCH)],
                                             start=(cb == 0), stop=(cb == 1))
                        nc.vector.tensor_scalar_add(f_rep[0:CQ, ts(fc, FCH)], pf, bf_sb)
                        pg = ltp.tile([CQ, FCH], FP32, tag="lt", name="pg")
                        for cb in range(2):
                            nc.tensor.matmul(pg, lhsT=wgT_b[:, cb, :],
                                             rhs=yb[:, cb, ts(fc, FCH)],
                                             start=(cb == 0), stop=(cb == 1))
                        nc.vector.tensor_scalar_add(g_rep[0:CQ, ts(fc, FCH)], pg, bg_sb)
                    # hhT for the j blocks covered by this chunk
                    for jb in range(ch * (DCH // 128), (ch + 1) * (DCH // 128)):
                        ph = utp.tile([128, C], FP32, tag="ut", name="ph")
                        nc.tensor.matmul(ph, lhsT=xb[:, 0, ts(jb, 128)], rhs=whT_b[:, 0, :],
                                         start=True, stop=False)
                        nc.tensor.matmul(ph, lhsT=xb[:, 1, ts(jb, 128)], rhs=whT_b[:, 1, :],
                                         start=False, stop=True)
                        nc.vector.tensor_add(hhT[:, jb, 0:C], ph, bh_bc)
                        nc.vector.memset(hhT[:, jb, C:C + 1], 1.0)
                # replicate the f/g bands in two halves (issued after all
                # input DMAs so the sync engine never stalls input issue)
                for h0 in (0, n // 2):
                    for r in range(1, 4):
                        nc.sync.dma_start(out=f_rep[32 * r:32 * (r + 1), ds(h0, n // 2)],
                                          in_=f_rep[0:32, ds(h0, n // 2)])
                        nc.sync.dma_start(out=g_rep[32 * r:32 * (r + 1), ds(h0, n // 2)],
                                          in_=g_rep[0:32, ds(h0, n // 2)])

                # ---- main attention loop -------------------------------

                JG2 = 8                  # j blocks per supergroup
                SGN = NB // JG2          # supergroups per i chunk

                def emit_pair(ic, sg, pr):
                    """2 row-packed logit matmuls into one [128, 2, IC]
                    psum pair tile + the exp over the pair; returns the bf16
                    exp tile covering jj = 2*pr, 2*pr+1 of the supergroup."""
                    lt = ltp.tile([128, 2, IC], FP32, tag="lt",
                                  name=f"lt{ic}_{sg}_{pr}")
                    for h in range(2):
                        jj = pr * 2 + h
                        j = sg * JG2 + jj
                        band = jj % 4
                        nc.tensor.matmul(
                            lt[:, h, :],
                            lhsT=g_rep[32 * band:32 * (band + 1), ts(j, 128)],
                            rhs=f_rep[32 * band:32 * (band + 1), ds(ic * IC, IC)],
                            start=True, stop=True,
                            tile_position=(32 * band, 0))
                    ex = exp_pool.tile([128, 2, IC], BF16, tag="ex",
                                       name=f"ex{ic}_{sg}_{pr}")
                    nc.scalar.activation(ex, lt, AF.Exp)
                    return ex

                def emit_ut(uts, exs, ic, sg, qs):
                    for q in qs:
                        for jj in range(JG2):
                            j = sg * JG2 + jj
                            nc.tensor.matmul(
                                uts[q][:, 0:C + 1],
                                lhsT=exs[jj // 2][:, jj % 2, ds(q * 128, 128)],
                                rhs=hhT[:, j, :],
                                start=(sg == 0 and jj == 0),
                                stop=(sg == SGN - 1 and jj == JG2 - 1),
                                skip_group_check=True)

                pending = {}
                for ic in range(NIC):
                    uts = [utp.tile([128, 512], FP32, tag="ut", name=f"ut{q}")
                           for q in range(NQ)]
                    if ic == 0:
                        pending[(0, 0)] = [emit_pair(0, 0, pr) for pr in range(4)]
                    for sg in range(SGN):
                        nxt = (ic, sg + 1) if sg + 1 < SGN else \
                              ((ic + 1, 0) if ic + 1 < NIC else None)
                        # pack A (jj 0-3) of the next supergroup ahead of this
                        # one's UT; pack B (jj 4-7) at the halfway point so
                        # the exps land just before the next UT run needs
                        # them (keeps ACT off the critical chain)
                        if nxt is not None:
                            pending[nxt] = [emit_pair(nxt[0], nxt[1], 0),
                                            emit_pair(nxt[0], nxt[1], 1)]
                        exs = pending.pop((ic, sg))
                        emit_ut(uts, exs, ic, sg, (0, 1))
                        if nxt is not None:
                            pending[nxt] += [emit_pair(nxt[0], nxt[1], 2),
                                             emit_pair(nxt[0], nxt[1], 3)]
                        emit_ut(uts, exs, ic, sg, (2, 3))
                    # tail: per q, normalize (freeing that ut psum bank),
                    # then emit the previous q's transpose/residual/store so
                    # the "ut" slot WAR chain resolves early and the PE is
                    # never starved across the i-chunk boundary.
                    satTs = []

                    def emit_xpose(qq):
                        i0 = ic * IC + qq * 128
                        stage = stagep.tile([128, 2, 128], FP32, tag="stage",
                                            name="stage")
                        tp = utp.tile([128, 128], BF16, tag="ut", name="tp")
                        for cb in range(2):
                            nc.tensor.transpose(tp, satTs[qq][:, ds(cb * 128, 128)],
                                                ident)
                            nc.vector.tensor_add(stage[:, cb, :], tp,
                                                 x_sb[:, cb, ds(i0, 128)])
                            nc.sync.dma_start(
                                out=out_d[cb * 128:(cb + 1) * 128, ds(i0, 128)],
                                in_=stage[:, cb, :])

                    for q in range(NQ):
                        rd = tailp.tile([128, 1], FP32, tag="rd")
                        nc.vector.reciprocal(rd, uts[q][:, C:C + 1])
                        satT = tailp.tile([128, C], BF16, tag="satT")
                        nc.vector.tensor_scalar(satT, uts[q][:, 0:C], rd, gamma_sb,
                                                op0=ALU.mult, op1=ALU.mult)
                        satTs.append(satT)
                        if q >= 1:
                            emit_xpose(q - 1)
                    emit_xpose(NQ - 1)

    nc.compile()
    return nc


_NC_CACHE = {}


def _get_nc(n=4096):
    if n not in _NC_CACHE:
        _NC_CACHE[n] = build_nc(n)
    return _NC_CACHE[n]


def make_in_maps(x, y, Wf, bf, Wg, bg, Wh, bh, gamma):
    x = np.asarray(x, dtype=np.float32)
    y = np.asarray(y, dtype=np.float32)
    B, C_, W_, H_ = x.shape
    n = W_ * H_
    wfT = np.ascontiguousarray(np.asarray(Wf, np.float32).T)
    wgT = np.ascontiguousarray(np.asarray(Wg, np.float32).T)
    whT = np.ascontiguousarray(np.asarray(Wh, np.float32).T)
    bf_ = np.asarray(bf, np.float32).reshape(CQ, 1)
    bg_ = np.asarray(bg, np.float32).reshape(CQ, 1)
    bh_ = np.asarray(bh, np.float32).reshape(1, C_)
    gm_ = np.asarray(gamma, np.float32).reshape(1, 1)
    in_maps = []
    for b in range(B):
        in_maps.append({
            "x": np.ascontiguousarray(x[b].reshape(C_, n)),
            "y": np.ascontiguousarray(y[b].reshape(C_, n)),
            "wfT": wfT, "wgT": wgT, "whT": whT,
            "bf": bf_, "bg": bg_, "bh": bh_, "gamma": gm_,
        })
    return in_maps, (B, C_, W_, H_)


def run_spmd(inputs: dict, trace: bool = False):
    """Run the SPMD kernel; returns (out [B,C,W,H], BassKernelResults)."""
    in_maps, (B, C_, W_, H_) = make_in_maps(**inputs)
    nc = _get_nc(W_ * H_)
    res = run_bass_kernel_spmd(nc, in_maps, core_ids=list(range(B)), trace=trace)
    out = np.stack([res.results[b]["out"].reshape(C_, W_, H_) for b in range(B)])
    return np.ascontiguousarray(out, dtype=np.float32), res


def kernel(x, y, Wf, bf, Wg, bg, Wh, bh, gamma):
    out, _ = run_spmd(dict(x=x, y=y, Wf=Wf, bf=bf, Wg=Wg, bg=bg,
                           Wh=Wh, bh=bh, gamma=gamma))
    return out
